# revision 22
# baseline (speedup 1.0000x reference)
"""Trainium2 Bass kernel for nn_Attention (B=4, S=2048, D=1024, DK=256).

Computation (reference, per batch b):
    qp = q @ Wq.T            [S, DK]
    kp = q @ Wk.T            [S, DK]
    scores = qp @ kp.T / sqrt(DK)
    attn = softmax(scores, axis=-1)
    out = attn @ q           (v = q)
    y = out @ Wv.T           [S, D]

Sharding: 8 cores = 4 batches x 2 query-halves. Each core handles one batch's
full key/value sequence and one 1024-row query half. The host "rolls" the
sequence per core so that the core's query half occupies rows 0..1023; since
softmax is invariant to key permutation this changes nothing numerically.

All matmul operands are bf16 (PSUM accumulation stays fp32; measured pipeline
rel-err ~4e-3 vs fp32 reference). bf16 halves input DMA to ~11MB/core, which
kills the startup DMA starvation the old fp32r version bridged with 48 dummy
warmup matmuls.

Per-core dataflow:
    inputs (host-packed bf16, 2-4KB DMA lines):
      qt  [16*128, 1024]  block (n,d): qT[d*128:+128, n*1024:+1024]
      wkq [4*128, 1024]   tile j cols: [wkT_d(2j) |wqT_d(2j) |wkT_d(2j+1)|wqT_d(2j+1)]
      qn  [8*128, 2048]   tile kk cols: [q_k(2kk) rows | q_k(2kk+1) rows] x D
      wvt [4*128, 2048]   tile j cols: [wvT_d(2j) 1024 | wvT_d(2j+1) 1024]
    kpT[e, s_k] = wkT.T @ qT          (per 512-col chunk, acc over d)
    qpT[e, s_q] = wqT.T @ qT[:, :1024]
    per s_q chunk of 512:
      scoresT[s_k, s_q] = kpT.T @ qpT   (16 k-tiles x 2 e-acc)
      expT = exp(scoresT / 16)          (ScalarE, PSUM->SBUF bf16, fused scale)
      denom: DVE pairwise tree (bf16 leaves -> fp32) -> ones-matmul partition
             sum -> PE-transpose 128-blocks -> reciprocal -> recip[s_q part, 1]
      unnormT[d, s_q] = qn.T @ expT     (8 d-tiles x 16 k-acc, 2 groups of 4)
      y[s_q, e_out] = unnormT.T @ wvT   (8 d-acc)
      y *= recip (per-partition) -> DMA out

DMA: inputs split over both HWDGE queues - SP carries wkq+qt (proj-phase
critical path), ACT carries qn+wvt (needed later); y outputs on SP.

PSUM discipline (8 banks): tag "acc" bufs=4 (qp accumulators, then unnorm
groups), tag "sc" bufs=3 (kp accumulators, score tiles, y tiles), "pd" 1.
"""

import numpy as np
import ml_dtypes

import concourse.mybir as mybir
import concourse.tile as tile
from concourse import bacc
from concourse.bass_utils import run_bass_kernel_spmd
from concourse.masks import make_identity

B, S, D, DK = 4, 2048, 1024, 256
SQ = S // 2  # query rows per core
P = 128
CH = 512  # s_q chunk width
NC = S // 512  # 4 proj column chunks
N_CORES = 8
WARMUP = 10

BF = mybir.dt.bfloat16
FR = mybir.dt.float32r
F32 = mybir.dt.float32
NPBF = ml_dtypes.bfloat16

KT = S // P  # 16 key tiles
DT = D // P  # 8 d tiles
ET = DK // P  # 2 e tiles

_PROGRAM = None


def _build_program():
    nc = bacc.Bacc(None, target_bir_lowering=False, debug=False)

    qt_d = nc.dram_tensor("qt", [DT * P, S], BF, kind="ExternalInput")
    wkq_d = nc.dram_tensor("wkq", [2 * P, 2048], BF, kind="ExternalInput")
    qn_d = nc.dram_tensor("qn", [4 * P, 4096], BF, kind="ExternalInput")
    wvt_d = nc.dram_tensor("wvt", [2 * P, 4096], BF, kind="ExternalInput")
    y_d = nc.dram_tensor("y", [SQ, D], F32, kind="ExternalOutput")

    with tile.TileContext(nc) as tc:
        with (
            tc.tile_pool(name="pp", bufs=1) as pp,
            tc.tile_pool(name="ps", bufs=1, space="PSUM") as ps,
        ):
            # ---- constants + warmup ----
            ones_f = pp.tile([P, 1], F32, tag="ones_f")
            nc.vector.memset(ones_f[:], 1.0)
            ones = pp.tile([P, 1], FR, tag="ones")
            nc.vector.tensor_copy(ones[:], ones_f[:])
            ident = pp.tile([P, P], F32, tag="ident")
            make_identity(nc, ident[:])
            # Warm the ACT exp table-set (~2.7us first-call cost) early.
            warm_act = pp.tile([P, 1], F32, tag="warm_act")
            nc.scalar.activation(
                warm_act[:], ones_f[:], mybir.ActivationFunctionType.Exp
            )
            # HAM warmup + boot->first-data bridge: dummy matmuls with no data
            # deps, covering the DMA-arming window until the first qt tile.
            warm_r = pp.tile([P, 512], BF, tag="warm_r")
            nc.vector.memset(warm_r[:], 1.0)
            pwarm = ps.tile([P, 512], F32, tag="sc", bufs=3, name="pwarm")
            for _ in range(WARMUP):
                nc.tensor.matmul(
                    pwarm[:], warm_r[:, :P], warm_r[:], start=True, stop=True
                )

            # ---- input tiles (everything SBUF-resident, 4KB+ DMA lines) ----
            wkq = [
                pp.tile([P, 2048], BF, tag="wkq", bufs=2, name=f"wkq{j}")
                for j in range(2)
            ]
            qt = [
                pp.tile([P, S], BF, tag="qt", bufs=DT, name=f"qt{d}")
                for d in range(DT)
            ]
            qn4 = [
                pp.tile([P, 4096], BF, tag="qn", bufs=4, name=f"qn{t}")
                for t in range(4)
            ]
            wv4 = [
                pp.tile([P, 4096], BF, tag="wvt", bufs=2, name=f"wv{j}")
                for j in range(2)
            ]

            # All input DMAs on the SP queue in strict consumption order
            # (ACT queue must stay clear: DMA triggers there would block the
            # exp activations behind them in the FIFO). wkq[0] goes in two
            # halves so the d=0/1 weights land with the first qt tile.
            nc.sync.dma_start(wkq[0][:, :1024], wkq_d[0:P, :1024])
            nc.sync.dma_start(qt[0][:], qt_d[0:P, :])
            nc.scalar.dma_start(qt[1][:], qt_d[P : 2 * P, :])
            nc.sync.dma_start(wkq[0][:, 1024:], wkq_d[0:P, 1024:])
            nc.scalar.dma_start(qt[3][:], qt_d[3 * P : 4 * P, :])
            nc.sync.dma_start(qt[2][:], qt_d[2 * P : 3 * P, :])
            nc.sync.dma_start(wkq[1][:], wkq_d[P : 2 * P, :])
            nc.sync.dma_start(qt[4][:], qt_d[4 * P : 5 * P, :])
            for d in range(5, DT):
                nc.sync.dma_start(qt[d][:], qt_d[d * P : (d + 1) * P, :])
            for t in range(4):
                nc.sync.dma_start(qn4[t][:], qn_d[t * P : (t + 1) * P, :])
            for j in range(2):
                nc.sync.dma_start(wv4[j][:], wvt_d[j * P : (j + 1) * P, :])

            # slicing helpers into the packed tiles
            def wk_sl(d, e):
                base = (d % 4) * 512 + e * P
                return wkq[d // 4][:, base : base + P]

            def wq_sl(d, e):
                base = (d % 4) * 512 + 256 + e * P
                return wkq[d // 4][:, base : base + P]

            def qt_sl(c):  # rhs [128, 512] per (chunk c, d)
                return lambda d: qt[d][:, c * 512 : (c + 1) * 512]

            def qn_sl(k, d):
                base = (k % 4) * 1024 + d * P
                return qn4[k // 4][:, base : base + P]

            def wv_sl(d, n):
                base = (d % 4) * 1024 + n * 512
                return wv4[d // 4][:, base : base + 512]

            # ---- persistent on-chip intermediates ----
            kpt = [
                pp.tile([P, S], BF, tag="kpt", bufs=ET, name=f"kpt{e}")
                for e in range(ET)
            ]
            qpt = {
                (e, c): pp.tile([P, CH], BF, tag="qpt", bufs=ET * 2, name=f"qpt{e}_{c}")
                for e in range(ET)
                for c in range(2)
            }
            expt = {}  # (chunk, k) -> bf16 tile, allocated on the fly

            # ---- helpers ----
            def scores_block(c, ks):
                """scoresT + exp for key tiles ks of chunk c."""
                for k in ks:
                    sc = ps.tile([P, CH], F32, tag="sc", bufs=3, name=f"sc{c}_{k}")
                    for e in range(ET):
                        nc.tensor.matmul(
                            sc[:],
                            kpt[e][:, k * P : (k + 1) * P],
                            qpt[e, c][:],
                            start=(e == 0),
                            stop=(e == ET - 1),
                        )
                    ex = pp.tile([P, CH], BF, tag="expt", bufs=20, name=f"ex{c}_{k}")
                    nc.scalar.activation(
                        ex[:], sc[:], mybir.ActivationFunctionType.Exp, scale=1.0 / 16.0
                    )
                    expt[c, k] = ex

            def proj_chunk(n, with_qp):
                """kp (and qp if with_qp) for qt column chunk n, acc over d.

                kp runs e-major with its PSUM->SBUF copy emitted right after
                each e finishes, so kpt lands early for the trailing scores.
                """
                rhs_of = qt_sl(n)
                for e in range(ET):
                    pk = ps.tile([P, 512], F32, tag="sc", bufs=3, name=f"pk{e}_{n}")
                    for d in range(DT):
                        nc.tensor.matmul(
                            pk[:],
                            wk_sl(d, e),
                            rhs_of(d),
                            start=(d == 0),
                            stop=(d == DT - 1),
                        )
                    nc.vector.tensor_copy(kpt[e][:, n * 512 : (n + 1) * 512], pk[:])
                if with_qp:
                    for e in range(ET):
                        pq = ps.tile([P, 512], F32, tag="acc", bufs=4, name=f"pq{e}_{n}")
                        for d in range(DT):
                            nc.tensor.matmul(
                                pq[:],
                                wq_sl(d, e),
                                rhs_of(d),
                                start=(d == 0),
                                stop=(d == DT - 1),
                            )
                        nc.vector.tensor_copy(qpt[e, n][:], pq[:])

            def denom_dve(c):
                """DVE part of the denominator: leaf adds pipelined with the
                exp stream, serial fp32 chain tracking them."""
                lvl = [
                    pp.tile([P, CH], F32, tag="dtree", bufs=8, name=f"dt{c}_{i}")
                    for i in range(8)
                ]
                for i in range(8):
                    nc.vector.tensor_tensor(
                        lvl[i][:],
                        expt[c, 2 * i][:],
                        expt[c, 2 * i + 1][:],
                        op=mybir.AluOpType.add,
                    )
                    if i > 0:
                        nc.vector.tensor_tensor(
                            lvl[0][:], lvl[0][:], lvl[i][:], op=mybir.AluOpType.add
                        )
                daccr = pp.tile([P, CH], FR, tag="daccr", bufs=2, name=f"daccr{c}")
                nc.vector.tensor_copy(daccr[:], lvl[0][:])
                return daccr

            def denom_pe(c, daccr):
                """PE part (partition-sum + transpose): emitted later in the
                PE stream so it never head-of-line blocks on the DVE tree."""
                pd = ps.tile([1, CH], F32, tag="pd", bufs=1, name=f"pd{c}")
                nc.tensor.matmul(pd[:], ones[:], daccr[:], start=True, stop=True)
                drow = pp.tile([1, CH], F32, tag="drow", bufs=2, name=f"drow{c}")
                nc.vector.tensor_copy(drow[:], pd[:])
                pt = ps.tile([P, CH // P], F32, tag="pd", bufs=1, name=f"pt{c}")
                for j in range(CH // P):
                    nc.tensor.transpose(
                        pt[:, j : j + 1], drow[:1, j * P : (j + 1) * P], ident[:1, :1]
                    )
                recip = pp.tile([P, CH // P], F32, tag="recip", bufs=2, name=f"recip{c}")
                nc.vector.reciprocal(recip[:], pt[:])
                return recip

            def unnorm_group(c, g, unsb):
                accs = [
                    ps.tile([P, CH], F32, tag="acc", bufs=4, name=f"un{c}_{g}_{i}")
                    for i in range(4)
                ]
                for k in range(KT):
                    for i in range(4):
                        d = g * 4 + i
                        nc.tensor.matmul(
                            accs[i][:],
                            qn_sl(k, d),
                            expt[c, k][:],
                            start=(k == 0),
                            stop=(k == KT - 1),
                        )
                for i in range(4):
                    us = pp.tile([P, CH], BF, tag="unsb", bufs=8, name=f"us{c}_{g}_{i}")
                    nc.vector.tensor_copy(us[:], accs[i][:])
                    unsb.append(us)

            def y_ms(c, unsb, recip, ms):
                cs = c * CH
                for m in ms:
                    for n in range(D // 512):
                        yb = ps.tile([P, 512], F32, tag="sc", bufs=3, name=f"yb{c}_{m}_{n}")
                        for d in range(DT):
                            nc.tensor.matmul(
                                yb[:],
                                unsb[d][:, m * P : (m + 1) * P],
                                wv_sl(d, n),
                                start=(d == 0),
                                stop=(d == DT - 1),
                            )
                        ys = pp.tile([P, 512], F32, tag="ysb", bufs=4, name=f"ys{c}_{m}_{n}")
                        nc.vector.tensor_scalar_mul(ys[:], yb[:], recip[:, m : m + 1])
                        nc.sync.dma_start(
                            y_d[cs + m * P : cs + (m + 1) * P, n * 512 : (n + 1) * 512],
                            ys[:],
                        )

            # ---- schedule (trace order == PE priority order) ----
            # proj chunks 0+1 back-to-back reuse the same qt tiles; their
            # PSUM->SBUF copies land under the following blocks, so no score
            # matmul ever head-of-line blocks the PE FIFO. Chunk-1 scores are
            # dosed between unnorm/y batches to stay within ACT's exp rate.
            proj_chunk(0, with_qp=True)
            proj_chunk(1, with_qp=True)
            scores_block(0, range(0, 4))
            proj_chunk(2, with_qp=False)
            scores_block(0, range(4, 8))
            proj_chunk(3, with_qp=False)
            scores_block(0, range(8, 12))
            scores_block(0, range(12, 16))
            daccr0 = denom_dve(0)
            unsb0 = []
            unnorm_group(0, 0, unsb0)
            scores_block(1, range(0, 4))
            recip0 = denom_pe(0, daccr0)
            unnorm_group(0, 1, unsb0)
            scores_block(1, range(4, 8))
            y_ms(0, unsb0, recip0, (0, 1))
            scores_block(1, range(8, 12))
            y_ms(0, unsb0, recip0, (2, 3))
            scores_block(1, range(12, 16))
            daccr1 = denom_dve(1)
            unsb1 = []
            unnorm_group(1, 0, unsb1)
            recip1 = denom_pe(1, daccr1)
            unnorm_group(1, 1, unsb1)
            y_ms(1, unsb1, recip1, (0, 1, 2, 3))

    nc.compile()
    return nc


def build_in_maps(q, Wq, Wk, Wv):
    q = np.asarray(q, dtype=np.float32)

    wqt = np.asarray(Wq, dtype=np.float32).T.astype(NPBF)  # [D, DK]
    wkt = np.asarray(Wk, dtype=np.float32).T.astype(NPBF)
    # wkq tile J: [128, 2048] cols (d%4)*512 + [wk 0:256 | wq 256:512], d=4J..4J+3
    wkq_full = np.concatenate([wkt, wqt], axis=1)  # [D, 512]
    wkq = np.ascontiguousarray(
        wkq_full.reshape(2, 4, P, 512).transpose(0, 2, 1, 3).reshape(2 * P, 2048)
    )
    # wvt tile J: [128, 4096] cols (d%4)*1024 + e, d=4J..4J+3
    wvT = np.asarray(Wv, dtype=np.float32).T.astype(NPBF)  # [D, D]
    wvt = np.ascontiguousarray(
        wvT.reshape(2, 4, P, 1024).transpose(0, 2, 1, 3).reshape(2 * P, 4096)
    )

    in_maps = []
    for core in range(N_CORES):
        b, h = divmod(core, 2)
        qb = q[b]
        rolled = np.concatenate(
            [qb[h * SQ : (h + 1) * SQ], qb[(1 - h) * SQ : (2 - h) * SQ]]
        ).astype(NPBF)
        # qt block d: qT[d*128:(d+1)*128, :] — qT row-blocks are contiguous
        qt_packed = np.ascontiguousarray(rolled.T)  # [D, S]
        # qn tile t: [128, 4096] cols (k%4)*1024 + d, k=4t..4t+3
        qn_packed = np.ascontiguousarray(
            rolled.reshape(4, 4, P, D).transpose(0, 2, 1, 3).reshape(4 * P, 4096)
        )
        in_maps.append(
            {
                "qt": qt_packed,
                "wkq": wkq,
                "qn": qn_packed,
                "wvt": wvt,
            }
        )
    return in_maps


def kernel(q, Wq, Wk, Wv):
    global _PROGRAM
    if _PROGRAM is None:
        _PROGRAM = _build_program()
    nc = _PROGRAM
    in_maps = build_in_maps(q, Wq, Wk, Wv)
    res = run_bass_kernel_spmd(nc, in_maps, list(range(N_CORES)))

    out = np.empty((B, S, D), dtype=np.float32)
    for core in range(N_CORES):
        b, h = divmod(core, 2)
        out[b, h * SQ : (h + 1) * SQ, :] = res.results[core]["y"]
    return out


# revision 23
# speedup vs baseline: 1.0081x; 1.0081x over previous
"""Trainium2 Bass kernel for nn_Attention (B=4, S=2048, D=1024, DK=256).

Computation (reference, per batch b):
    qp = q @ Wq.T            [S, DK]
    kp = q @ Wk.T            [S, DK]
    scores = qp @ kp.T / sqrt(DK)
    attn = softmax(scores, axis=-1)
    out = attn @ q           (v = q)
    y = out @ Wv.T           [S, D]

Sharding: 8 cores = 4 batches x 2 query-halves. Each core handles one batch's
full key/value sequence and one 1024-row query half. The host "rolls" the
sequence per core so that the core's query half occupies rows 0..1023; since
softmax is invariant to key permutation this changes nothing numerically.

All matmul operands are bf16 (PSUM accumulation stays fp32; measured pipeline
rel-err ~4e-3 vs fp32 reference). bf16 halves input DMA to ~11MB/core, which
kills the startup DMA starvation the old fp32r version bridged with 48 dummy
warmup matmuls.

Per-core dataflow:
    inputs (host-packed bf16, 2-4KB DMA lines):
      qt  [16*128, 1024]  block (n,d): qT[d*128:+128, n*1024:+1024]
      wkq [4*128, 1024]   tile j cols: [wkT_d(2j) |wqT_d(2j) |wkT_d(2j+1)|wqT_d(2j+1)]
      qn  [8*128, 2048]   tile kk cols: [q_k(2kk) rows | q_k(2kk+1) rows] x D
      wvt [4*128, 2048]   tile j cols: [wvT_d(2j) 1024 | wvT_d(2j+1) 1024]
    kpT[e, s_k] = wkT.T @ qT          (per 512-col chunk, acc over d)
    qpT[e, s_q] = wqT.T @ qT[:, :1024]
    per s_q chunk of 512:
      scoresT[s_k, s_q] = kpT.T @ qpT   (16 k-tiles x 2 e-acc)
      expT = exp(scoresT / 16)          (ScalarE, PSUM->SBUF bf16, fused scale)
      denom: DVE pairwise tree (bf16 leaves -> fp32) -> ones-matmul partition
             sum -> PE-transpose 128-blocks -> reciprocal -> recip[s_q part, 1]
      unnormT[d, s_q] = qn.T @ expT     (8 d-tiles x 16 k-acc, 2 groups of 4)
      y[s_q, e_out] = unnormT.T @ wvT   (8 d-acc)
      y *= recip (per-partition) -> DMA out

DMA: inputs split over both HWDGE queues - SP carries wkq+qt (proj-phase
critical path), ACT carries qn+wvt (needed later); y outputs on SP.

PSUM discipline (8 banks): tag "acc" bufs=4 (qp accumulators, then unnorm
groups), tag "sc" bufs=3 (kp accumulators, score tiles, y tiles), "pd" 1.
"""

import numpy as np
import ml_dtypes

import concourse.mybir as mybir
import concourse.tile as tile
from concourse import bacc
from concourse.bass_utils import run_bass_kernel_spmd
from concourse.masks import make_identity

B, S, D, DK = 4, 2048, 1024, 256
SQ = S // 2  # query rows per core
P = 128
CH = 512  # s_q chunk width
NC = S // 512  # 4 proj column chunks
N_CORES = 8
WARMUP = 13

BF = mybir.dt.bfloat16
FR = mybir.dt.float32r
F32 = mybir.dt.float32
NPBF = ml_dtypes.bfloat16

KT = S // P  # 16 key tiles
DT = D // P  # 8 d tiles
ET = DK // P  # 2 e tiles

_PROGRAM = None


def _build_program():
    nc = bacc.Bacc(None, target_bir_lowering=False, debug=False)

    qt_d = nc.dram_tensor("qt", [DT * P, S], BF, kind="ExternalInput")
    wkq_d = nc.dram_tensor("wkq", [2 * P, 2048], BF, kind="ExternalInput")
    qn_d = nc.dram_tensor("qn", [4 * P, 4096], BF, kind="ExternalInput")
    wvt_d = nc.dram_tensor("wvt", [2 * P, 4096], BF, kind="ExternalInput")
    y_d = nc.dram_tensor("y", [SQ, D], F32, kind="ExternalOutput")

    with tile.TileContext(nc) as tc:
        with (
            tc.tile_pool(name="pp", bufs=1) as pp,
            tc.tile_pool(name="ps", bufs=1, space="PSUM") as ps,
        ):
            # ---- constants + warmup ----
            ones_f = pp.tile([P, 1], F32, tag="ones_f")
            nc.vector.memset(ones_f[:], 1.0)
            ones = pp.tile([P, 1], FR, tag="ones")
            nc.vector.tensor_copy(ones[:], ones_f[:])
            ident = pp.tile([P, P], F32, tag="ident")
            make_identity(nc, ident[:])
            # Warm the ACT exp table-set (~2.7us first-call cost) early.
            warm_act = pp.tile([P, 1], F32, tag="warm_act")
            nc.scalar.activation(
                warm_act[:], ones_f[:], mybir.ActivationFunctionType.Exp
            )
            # HAM warmup + boot->first-data bridge: dummy matmuls with no data
            # deps, covering the DMA-arming window until the first qt tile.
            warm_r = pp.tile([P, 512], BF, tag="warm_r")
            nc.vector.memset(warm_r[:], 1.0)
            pwarm = ps.tile([P, 512], F32, tag="sc", bufs=3, name="pwarm")
            for _ in range(WARMUP):
                nc.tensor.matmul(
                    pwarm[:], warm_r[:, :P], warm_r[:], start=True, stop=True
                )

            # ---- input tiles (everything SBUF-resident, 4KB+ DMA lines) ----
            wkq = [
                pp.tile([P, 2048], BF, tag="wkq", bufs=2, name=f"wkq{j}")
                for j in range(2)
            ]
            qt = [
                pp.tile([P, S], BF, tag="qt", bufs=DT, name=f"qt{d}")
                for d in range(DT)
            ]
            qn4 = [
                pp.tile([P, 4096], BF, tag="qn", bufs=4, name=f"qn{t}")
                for t in range(4)
            ]
            wv4 = [
                pp.tile([P, 4096], BF, tag="wvt", bufs=2, name=f"wv{j}")
                for j in range(2)
            ]

            # All input DMAs on the SP queue in strict consumption order
            # (ACT queue must stay clear: DMA triggers there would block the
            # exp activations behind them in the FIFO). wkq[0] goes in two
            # halves so the d=0/1 weights land with the first qt tile.
            nc.sync.dma_start(wkq[0][:, :1024], wkq_d[0:P, :1024])
            nc.sync.dma_start(qt[0][:], qt_d[0:P, :])
            nc.scalar.dma_start(qt[1][:], qt_d[P : 2 * P, :])
            nc.sync.dma_start(wkq[0][:, 1024:], wkq_d[0:P, 1024:])
            nc.scalar.dma_start(qt[3][:], qt_d[3 * P : 4 * P, :])
            nc.sync.dma_start(qt[2][:], qt_d[2 * P : 3 * P, :])
            nc.sync.dma_start(wkq[1][:], wkq_d[P : 2 * P, :])
            nc.sync.dma_start(qt[4][:], qt_d[4 * P : 5 * P, :])
            for d in range(5, DT):
                nc.sync.dma_start(qt[d][:], qt_d[d * P : (d + 1) * P, :])
            for t in range(4):
                nc.sync.dma_start(qn4[t][:], qn_d[t * P : (t + 1) * P, :])
            for j in range(2):
                nc.sync.dma_start(wv4[j][:], wvt_d[j * P : (j + 1) * P, :])

            # slicing helpers into the packed tiles
            def wk_sl(d, e):
                base = (d % 4) * 512 + e * P
                return wkq[d // 4][:, base : base + P]

            def wq_sl(d, e):
                base = (d % 4) * 512 + 256 + e * P
                return wkq[d // 4][:, base : base + P]

            def qt_sl(c):  # rhs [128, 512] per (chunk c, d)
                return lambda d: qt[d][:, c * 512 : (c + 1) * 512]

            def qn_sl(k, d):
                base = (k % 4) * 1024 + d * P
                return qn4[k // 4][:, base : base + P]

            def wv_sl(d, n):
                base = (d % 4) * 1024 + n * 512
                return wv4[d // 4][:, base : base + 512]

            # ---- persistent on-chip intermediates ----
            kpt = [
                pp.tile([P, S], BF, tag="kpt", bufs=ET, name=f"kpt{e}")
                for e in range(ET)
            ]
            qpt = {
                (e, c): pp.tile([P, CH], BF, tag="qpt", bufs=ET * 2, name=f"qpt{e}_{c}")
                for e in range(ET)
                for c in range(2)
            }
            expt = {}  # (chunk, k) -> bf16 tile, allocated on the fly

            # ---- helpers ----
            def scores_block(c, ks):
                """scoresT + exp for key tiles ks of chunk c."""
                for k in ks:
                    sc = ps.tile([P, CH], F32, tag="sc", bufs=3, name=f"sc{c}_{k}")
                    for e in range(ET):
                        nc.tensor.matmul(
                            sc[:],
                            kpt[e][:, k * P : (k + 1) * P],
                            qpt[e, c][:],
                            start=(e == 0),
                            stop=(e == ET - 1),
                        )
                    ex = pp.tile([P, CH], BF, tag="expt", bufs=20, name=f"ex{c}_{k}")
                    nc.scalar.activation(
                        ex[:], sc[:], mybir.ActivationFunctionType.Exp, scale=1.0 / 16.0
                    )
                    expt[c, k] = ex

            def proj_chunk(n, with_qp):
                """kp (and qp if with_qp) for qt column chunk n, acc over d.

                kp runs e-major with its PSUM->SBUF copy emitted right after
                each e finishes, so kpt lands early for the trailing scores.
                """
                rhs_of = qt_sl(n)
                for e in range(ET):
                    pk = ps.tile([P, 512], F32, tag="sc", bufs=3, name=f"pk{e}_{n}")
                    for d in range(DT):
                        nc.tensor.matmul(
                            pk[:],
                            wk_sl(d, e),
                            rhs_of(d),
                            start=(d == 0),
                            stop=(d == DT - 1),
                        )
                    nc.vector.tensor_copy(kpt[e][:, n * 512 : (n + 1) * 512], pk[:])
                if with_qp:
                    for e in range(ET):
                        pq = ps.tile([P, 512], F32, tag="acc", bufs=4, name=f"pq{e}_{n}")
                        for d in range(DT):
                            nc.tensor.matmul(
                                pq[:],
                                wq_sl(d, e),
                                rhs_of(d),
                                start=(d == 0),
                                stop=(d == DT - 1),
                            )
                        nc.vector.tensor_copy(qpt[e, n][:], pq[:])

            def denom_dve(c):
                """DVE part of the denominator: leaf adds pipelined with the
                exp stream, serial fp32 chain tracking them."""
                lvl = [
                    pp.tile([P, CH], F32, tag="dtree", bufs=8, name=f"dt{c}_{i}")
                    for i in range(8)
                ]
                for i in range(8):
                    nc.vector.tensor_tensor(
                        lvl[i][:],
                        expt[c, 2 * i][:],
                        expt[c, 2 * i + 1][:],
                        op=mybir.AluOpType.add,
                    )
                    if i > 0:
                        nc.vector.tensor_tensor(
                            lvl[0][:], lvl[0][:], lvl[i][:], op=mybir.AluOpType.add
                        )
                daccr = pp.tile([P, CH], FR, tag="daccr", bufs=2, name=f"daccr{c}")
                nc.vector.tensor_copy(daccr[:], lvl[0][:])
                return daccr

            def denom_pe(c, daccr):
                """PE part (partition-sum + transpose): emitted later in the
                PE stream so it never head-of-line blocks on the DVE tree."""
                pd = ps.tile([1, CH], F32, tag="pd", bufs=1, name=f"pd{c}")
                nc.tensor.matmul(pd[:], ones[:], daccr[:], start=True, stop=True)
                drow = pp.tile([1, CH], F32, tag="drow", bufs=2, name=f"drow{c}")
                nc.vector.tensor_copy(drow[:], pd[:])
                pt = ps.tile([P, CH // P], F32, tag="pd", bufs=1, name=f"pt{c}")
                for j in range(CH // P):
                    nc.tensor.transpose(
                        pt[:, j : j + 1], drow[:1, j * P : (j + 1) * P], ident[:1, :1]
                    )
                recip = pp.tile([P, CH // P], F32, tag="recip", bufs=2, name=f"recip{c}")
                nc.vector.reciprocal(recip[:], pt[:])
                return recip

            def unnorm_group(c, g, unsb):
                accs = [
                    ps.tile([P, CH], F32, tag="acc", bufs=4, name=f"un{c}_{g}_{i}")
                    for i in range(4)
                ]
                for k in range(KT):
                    for i in range(4):
                        d = g * 4 + i
                        nc.tensor.matmul(
                            accs[i][:],
                            qn_sl(k, d),
                            expt[c, k][:],
                            start=(k == 0),
                            stop=(k == KT - 1),
                        )
                for i in range(4):
                    us = pp.tile([P, CH], BF, tag="unsb", bufs=8, name=f"us{c}_{g}_{i}")
                    nc.vector.tensor_copy(us[:], accs[i][:])
                    unsb.append(us)

            def y_ms(c, unsb, recip, ms):
                cs = c * CH
                for m in ms:
                    for n in range(D // 512):
                        yb = ps.tile([P, 512], F32, tag="sc", bufs=3, name=f"yb{c}_{m}_{n}")
                        for d in range(DT):
                            nc.tensor.matmul(
                                yb[:],
                                unsb[d][:, m * P : (m + 1) * P],
                                wv_sl(d, n),
                                start=(d == 0),
                                stop=(d == DT - 1),
                            )
                        ys = pp.tile([P, 512], F32, tag="ysb", bufs=4, name=f"ys{c}_{m}_{n}")
                        nc.vector.tensor_scalar_mul(ys[:], yb[:], recip[:, m : m + 1])
                        nc.sync.dma_start(
                            y_d[cs + m * P : cs + (m + 1) * P, n * 512 : (n + 1) * 512],
                            ys[:],
                        )

            # ---- schedule (trace order == PE priority order) ----
            # proj chunks 0+1 back-to-back reuse the same qt tiles; their
            # PSUM->SBUF copies land under the following blocks, so no score
            # matmul ever head-of-line blocks the PE FIFO. Chunk-1 scores are
            # dosed between unnorm/y batches to stay within ACT's exp rate.
            proj_chunk(0, with_qp=True)
            proj_chunk(1, with_qp=True)
            scores_block(0, range(0, 4))
            proj_chunk(2, with_qp=False)
            scores_block(0, range(4, 8))
            proj_chunk(3, with_qp=False)
            scores_block(0, range(8, 12))
            scores_block(0, range(12, 16))
            daccr0 = denom_dve(0)
            unsb0 = []
            unnorm_group(0, 0, unsb0)
            scores_block(1, range(0, 4))
            recip0 = denom_pe(0, daccr0)
            unnorm_group(0, 1, unsb0)
            scores_block(1, range(4, 8))
            y_ms(0, unsb0, recip0, (0, 1))
            scores_block(1, range(8, 12))
            y_ms(0, unsb0, recip0, (2, 3))
            scores_block(1, range(12, 16))
            daccr1 = denom_dve(1)
            unsb1 = []
            unnorm_group(1, 0, unsb1)
            recip1 = denom_pe(1, daccr1)
            unnorm_group(1, 1, unsb1)
            y_ms(1, unsb1, recip1, (0, 1, 2, 3))

    nc.compile()
    return nc


def build_in_maps(q, Wq, Wk, Wv):
    q = np.asarray(q, dtype=np.float32)

    wqt = np.asarray(Wq, dtype=np.float32).T.astype(NPBF)  # [D, DK]
    wkt = np.asarray(Wk, dtype=np.float32).T.astype(NPBF)
    # wkq tile J: [128, 2048] cols (d%4)*512 + [wk 0:256 | wq 256:512], d=4J..4J+3
    wkq_full = np.concatenate([wkt, wqt], axis=1)  # [D, 512]
    wkq = np.ascontiguousarray(
        wkq_full.reshape(2, 4, P, 512).transpose(0, 2, 1, 3).reshape(2 * P, 2048)
    )
    # wvt tile J: [128, 4096] cols (d%4)*1024 + e, d=4J..4J+3
    wvT = np.asarray(Wv, dtype=np.float32).T.astype(NPBF)  # [D, D]
    wvt = np.ascontiguousarray(
        wvT.reshape(2, 4, P, 1024).transpose(0, 2, 1, 3).reshape(2 * P, 4096)
    )

    in_maps = []
    for core in range(N_CORES):
        b, h = divmod(core, 2)
        qb = q[b]
        rolled = np.concatenate(
            [qb[h * SQ : (h + 1) * SQ], qb[(1 - h) * SQ : (2 - h) * SQ]]
        ).astype(NPBF)
        # qt block d: qT[d*128:(d+1)*128, :] — qT row-blocks are contiguous
        qt_packed = np.ascontiguousarray(rolled.T)  # [D, S]
        # qn tile t: [128, 4096] cols (k%4)*1024 + d, k=4t..4t+3
        qn_packed = np.ascontiguousarray(
            rolled.reshape(4, 4, P, D).transpose(0, 2, 1, 3).reshape(4 * P, 4096)
        )
        in_maps.append(
            {
                "qt": qt_packed,
                "wkq": wkq,
                "qn": qn_packed,
                "wvt": wvt,
            }
        )
    return in_maps


def kernel(q, Wq, Wk, Wv):
    global _PROGRAM
    if _PROGRAM is None:
        _PROGRAM = _build_program()
    nc = _PROGRAM
    in_maps = build_in_maps(q, Wq, Wk, Wv)
    res = run_bass_kernel_spmd(nc, in_maps, list(range(N_CORES)))

    out = np.empty((B, S, D), dtype=np.float32)
    for core in range(N_CORES):
        b, h = divmod(core, 2)
        out[b, h * SQ : (h + 1) * SQ, :] = res.results[core]["y"]
    return out
